# revision 5
# baseline (speedup 1.0000x reference)
"""DETM decoder kernel for 8 Trainium2 NeuronCores (L-chunk-outer matmul order).

Combines v2's instruction-count cuts with v3's locality reordering:
  - phase 1 iterates 2 v-blocks (4096 + 2560 cols) outermost, topics
    inner, so each rhoT slice is streamed from (host) memory twice per
    rep instead of 50 times; full-PSUM tile per (topic, vblock) with a
    single exp activation (100 total).
  - E stash in 5-topic staging tiles: 20 stash DMAs; phase 2 reloads in
    5-topic tiles: 10 loads + 50 weighted-accumulate DVE ops.
  - Z via one [128,50] AllReduce as before.
~2.35k instructions/core vs 2.73k baseline, 2.43k v2.
"""
import sys
import functools

if "/opt/trn_rl_repo" not in sys.path:
    sys.path.insert(0, "/opt/trn_rl_repo")

import numpy as np

from concourse import bacc, mybir, bass_utils
import concourse.tile as tile

B, K, L, V = 128, 50, 300, 50000
NCORES = 8
VC = 6656                 # padded columns per core (13 x 512)
VPAD = VC * NCORES        # 53248
ROWS = K * B              # 6400 (row = k*128 + b)
G = 5                     # topics per stash staging tile
GL = 5                    # topics per phase-2 reload tile

F32 = mybir.dt.float32
F32R = mybir.dt.float32r
BF16 = mybir.dt.bfloat16
Exp = mybir.ActivationFunctionType.Exp

LCHUNKS = [(0, 128), (128, 128), (256, 44)]          # L = 300
VBLOCKS = [(0, 4096), (4096, 2560)]                  # VC = 6656


def _body(nc, tc, dram, io):
    """One full computation of the output (repeatable for timing)."""
    from contextlib import ExitStack

    with ExitStack() as ctx:
        res = ctx.enter_context(tc.tile_pool(name="res", bufs=1))

        cneg_sb = res.tile([B, K], F32)
        nc.sync.dma_start(cneg_sb[:], io["cneg"])
        th_sb = res.tile([B, K], F32)
        nc.sync.dma_start(th_sb[:], io["thetaT"])
        zp = res.tile([B, K * 2], F32)        # per-(k, vblock) partial Z
        zall = res.tile([B, K], F32)
        rz = res.tile([B, K], F32)
        w_sb = res.tile([B, K], F32)

        pstash = dram.tile([K, B, VC], BF16)
        zin_d = dram.tile([B, K], F32)
        zout_d = dram.tile([B, K], F32)

        # ========== phase 1: logits -> exp -> stash + Z (vblock outer) =====
        with tc.tile_pool(name="mats", bufs=1) as mats, \
             tc.tile_pool(name="ps1", bufs=1, space="PSUM") as ps1, \
             tc.tile_pool(name="pstage", bufs=1) as pstage:
            at_sb = []
            rt_sb = []
            for lc, (l0, ln) in enumerate(LCHUNKS):
                t = mats.tile([ln, ROWS], F32R, tag=f"at{lc}", name=f"at{lc}")
                nc.sync.dma_start(t[:], io["alphaT"][l0:l0 + ln, :])
                at_sb.append(t)
                r = mats.tile([ln, VC], F32R, tag=f"rt{lc}", name=f"rt{lc}")
                nc.sync.dma_start(r[:], io["rhoT"][l0:l0 + ln, :])
                rt_sb.append(r)

            for vbi, (v0, vn) in enumerate(VBLOCKS):
                for kg in range(K // G):
                    pst = pstage.tile([B, G * vn], BF16, tag="pst",
                                      name="pst")
                    for g in range(G):
                        k = kg * G + g
                        kcol = slice(k * B, (k + 1) * B)
                        psh = ps1.tile([B, vn], F32, tag="psh", name="psh")
                        for lc in range(3):
                            for j in range(vn // 512):
                                nc.tensor.matmul(
                                    psh[:, j * 512:(j + 1) * 512],
                                    at_sb[lc][:, kcol],
                                    rt_sb[lc][:, v0 + j * 512:
                                              v0 + (j + 1) * 512],
                                    start=(lc == 0), stop=(lc == 2))
                        nc.scalar.activation(
                            pst[:, g * vn:(g + 1) * vn], psh[:], Exp,
                            bias=cneg_sb[:, k:k + 1], scale=1.0,
                            accum_out=zp[:, k * 2 + vbi:k * 2 + vbi + 1])
                    nc.sync.dma_start(
                        pstash[kg * G:(kg + 1) * G, :, v0:v0 + vn]
                        .rearrange("g b v -> b g v"),
                        pst[:].rearrange("b (g v) -> b g v", g=G))

        # ================= Z allreduce + weights ===========================
        zv = zp[:].rearrange("p (k t) -> p k t", k=K)
        nc.vector.tensor_reduce(zall[:], zv, axis=mybir.AxisListType.X,
                                op=mybir.AluOpType.add)
        nc.sync.dma_start(zin_d[:], zall[:])
        nc.gpsimd.collective_compute(
            "AllReduce", mybir.AluOpType.add,
            replica_groups=[list(range(NCORES))],
            ins=[zin_d.opt()], outs=[zout_d.opt()])
        nc.sync.dma_start(zall[:], zout_d[:])
        nc.vector.reciprocal(rz[:], zall[:])
        nc.vector.tensor_mul(w_sb[:], th_sb[:], rz[:])

        # ========= phase 2: out[b,v] = sum_k w[k,b] * P_k[b,v] =============
        with tc.tile_pool(name="ld", bufs=2) as ldp, \
             tc.tile_pool(name="accp", bufs=1) as accp:
            acc = accp.tile([B, VC], F32)
            for p in range(K // GL):
                ld = ldp.tile([B, GL * VC], BF16, tag="ld", name="ld")
                nc.sync.dma_start(
                    ld[:].rearrange("b (t v) -> b t v", t=GL),
                    pstash[p * GL:(p + 1) * GL].rearrange("t b v -> b t v"))
                for t in range(GL):
                    k = p * GL + t
                    lslc = ld[:, t * VC:(t + 1) * VC]
                    if k == 0:
                        nc.vector.tensor_scalar_mul(acc[:], lslc,
                                                    w_sb[:, 0:1])
                    else:
                        nc.vector.scalar_tensor_tensor(
                            acc[:], lslc, w_sb[:, k:k + 1], acc[:],
                            op0=mybir.AluOpType.mult, op1=mybir.AluOpType.add)
            nc.sync.dma_start(io["out"], acc[:])


@functools.lru_cache(maxsize=2)
def _build(reps=1):
    nc = bacc.Bacc("TRN2", target_bir_lowering=False, debug=False,
                   num_devices=NCORES)
    io = {
        "alphaT": nc.dram_tensor("alphaT", [L, ROWS], F32R,
                                 kind="ExternalInput").ap(),
        "rhoT": nc.dram_tensor("rhoT", [L, VC], F32R,
                               kind="ExternalInput").ap(),
        "cneg": nc.dram_tensor("cneg", [B, K], F32,
                               kind="ExternalInput").ap(),
        "thetaT": nc.dram_tensor("thetaT", [B, K], F32,
                                 kind="ExternalInput").ap(),
        "out": nc.dram_tensor("out", [B, VC], F32,
                              kind="ExternalOutput").ap(),
    }
    with tile.TileContext(nc) as tc:
        with tc.tile_pool(name="dram", bufs=1, space="DRAM") as dram:
            for _ in range(reps):
                _body(nc, tc, dram, io)
    nc.compile()
    return nc, io


def _host_prep(theta, alpha, word_embeddings):
    theta = np.ascontiguousarray(theta, dtype=np.float32)
    alpha = np.ascontiguousarray(alpha, dtype=np.float32)
    we = np.ascontiguousarray(word_embeddings, dtype=np.float32)

    alphaT = np.ascontiguousarray(
        alpha.transpose(2, 1, 0).reshape(L, ROWS))       # col = k*128 + b
    rhoT = np.zeros((L, VPAD), np.float32)
    rhoT[:, :V] = we.T
    # per-(b,k) safe shift: statistical upper bound on max_v logits
    cneg = -(4.65 * np.linalg.norm(alpha, axis=2) + 10.0).astype(np.float32)

    in_maps = []
    for c in range(NCORES):
        in_maps.append({
            "alphaT": alphaT,
            "rhoT": np.ascontiguousarray(rhoT[:, c * VC:(c + 1) * VC]),
            "cneg": cneg,
            "thetaT": theta,
        })
    return in_maps


def run_on_cores(theta, alpha, word_embeddings, reps=1):
    nc, io = _build(reps)
    in_maps = _host_prep(theta, alpha, word_embeddings)
    res = bass_utils.run_bass_kernel_spmd(nc, in_maps,
                                          core_ids=list(range(NCORES)))
    out = np.concatenate([res.results[c]["out"] for c in range(NCORES)],
                         axis=1)
    return out[:, :V].astype(np.float32)


def kernel(theta, alpha, word_embeddings):
    return run_on_cores(theta, alpha, word_embeddings, reps=1)
